# revision 7
# baseline (speedup 1.0000x reference)
"""Trainium2 Bass kernel v2 for nn_BertFreezeSegmentor (BiLSTM + stack-decoder).

Restructure vs v1 baseline:
  - Two pipelined phases instead of six serial ones:
      scan phase:   fwd+bwd LSTM scans interleaved per slot, XF/XB gate
                    GEMMs computed block-by-block in the PE gaps, h history
                    streamed to DRAM per 16-step block, CX classifier GEMM
                    computed as soon as both directions' blocks land.
      decode phase: subword+word chains interleaved (word lags LAG steps),
                    SD / WI / cls GEMMs computed block-by-block in the PE
                    gaps from streamed inputs; swih/wwih streamed from DRAM
                    in (quarter, k) chunks so everything fits SBUF.
  - Per-step critical path shortened: the x-projection (incl. bias) is
    added into PSUM with identity-stationary matmuls so the activations
    read PSUM directly; gates reordered to [i, f, o, g] so one Sigmoid
    covers i,f,o; h written directly into history block tiles.

Sharding: pure data parallelism, 8 examples per core on 8 cores.
"""

import numpy as np
import ml_dtypes

import concourse.bass as bass
import concourse.tile as tile
from concourse import bacc, mybir
from concourse.bass_utils import run_bass_kernel_spmd

BF16 = ml_dtypes.bfloat16
DT_BF = mybir.dt.bfloat16
DT_F32 = mybir.dt.float32
AF = mybir.ActivationFunctionType

FULL = dict(S=256, B=8, H=768, NCORES=8)


def build_program(S, B, H, num_devices=8, phases="SD"):
    CH = H // 128           # h chunks (6)
    GM = 4 * H // 128       # gate m-tiles (24)
    C2 = 2 * H // 128       # [h;c] chunks (12)
    NC = S * B
    KB = 16                 # scan block steps
    DB = 32                 # decode block steps
    KC = KB * B
    DC = DB * B
    NKB = S // KB
    NDB = S // DB
    LAG = 40                # word chain lag (steps) behind subword
    assert S % KB == 0 and S % DB == 0

    nc = bacc.Bacc("TRN2", target_bir_lowering=False, debug=False,
                   enable_asserts=False, num_devices=num_devices)

    def inp(name, shape, dt):
        return nc.dram_tensor(name, shape, dt, kind="ExternalInput").ap()

    def scratch(name, shape, dt):
        return nc.dram_tensor(name, shape, dt, kind="Internal").ap()

    def outp(name, shape, dt):
        return nc.dram_tensor(name, shape, dt, kind="ExternalOutput").ap()

    # ---- inputs ----
    xT = inp("xT", [128, CH, NC], DT_BF)
    xTr = inp("xTr", [128, CH, NC], DT_BF)
    wih_f = inp("wih_f", [128, CH, 4 * H], DT_BF)
    whh_f = inp("whh_f", [128, CH, 4 * H], DT_BF)
    wih_b = inp("wih_b", [128, CH, 4 * H], DT_BF)
    whh_b = inp("whh_b", [128, CH, 4 * H], DT_BF)
    bias_f = inp("bias_f", [1, 4 * H], DT_BF)
    bias_b = inp("bias_b", [1, 4 * H], DT_BF)
    swih3 = inp("swih3", [128, C2, 8, 384], DT_BF)   # (k, m-triple) chunks
    swhh = inp("swhh", [128, CH, 4 * H], DT_BF)
    sbias = inp("sbias", [1, 4 * H], DT_BF)
    wwih3 = inp("wwih3", [128, C2, 8, 384], DT_BF)
    wwhh = inp("wwhh", [128, CH, 4 * H], DT_BF)
    wbias = inp("wbias", [1, 4 * H], DT_BF)
    cls1T = inp("cls1T", [128, CH, 2], DT_BF)
    cls2T = inp("cls2T", [128, C2, 2], DT_BF)
    keep6 = inp("keep6", [128, CH, NC], DT_BF)
    wsel6 = inp("wsel6", [128, CH, NC], DT_BF)
    ident = inp("ident", [128, 128], DT_BF)

    # ---- DRAM scratch: h histories by t ----
    HF_D = scratch("HF_D", [128, CH, NC], DT_BF)
    HB_D = scratch("HB_D", [128, CH, NC], DT_BF)
    WH_D = scratch("WH_D", [128, CH, NC], DT_BF)

    # ---- outputs ----
    cx_t = outp("cx_t", [2, NC], DT_F32)
    wcls_t = outp("wcls_t", [2, NC], DT_F32)

    with tile.TileContext(nc) as tc:
        _dma_rr = [0]

        def dma_eng():
            _dma_rr[0] += 1
            return nc.sync if _dma_rr[0] % 2 else nc.gpsimd

        def load_w(pool, src, tag):
            t = pool.tile(list(src.shape), src.dtype, tag=tag, name=tag)
            if len(src.shape) >= 3 and src.shape[1] > 1:
                for k in range(src.shape[1]):
                    dma_eng().dma_start(t[:, k], src[:, k])
            else:
                dma_eng().dma_start(t[:], src[:])
            return t

        with tc.tile_pool(name="const", bufs=1) as cpool:
            id_sb = load_w(cpool, ident, "id_sb")
            ones = cpool.tile([1, DC], DT_BF, tag="ones", name="ones")
            nc.vector.memset(ones[:], 1.0)
            zhc = cpool.tile([128, CH, B], DT_BF, tag="zhc", name="zhc")
            nc.vector.memset(zhc[:], 0.0)

            def step_tail(sp, ps, cprev, hout_ap, cf_out, dd):
                """gates psum [128, GM, B] (order i,f,o,g) -> h, c."""
                sif = sp.tile([128, 3 * CH, B], DT_F32, tag=f"sif{dd}")
                nc.scalar.activation(sif[:], ps[:, 0:3 * CH, :], AF.Sigmoid)
                tg = sp.tile([128, CH, B], DT_F32, tag=f"tg{dd}")
                nc.scalar.activation(tg[:], ps[:, 3 * CH:4 * CH, :], AF.Tanh)
                t1 = sp.tile([128, CH, B], DT_F32, tag=f"t1{dd}")
                nc.vector.tensor_mul(t1[:], sif[:, CH:2 * CH, :], cprev[:])
                t2 = sp.tile([128, CH, B], DT_F32, tag=f"t2{dd}")
                nc.vector.tensor_mul(t2[:], sif[:, 0:CH, :], tg[:])
                nc.vector.tensor_add(cf_out[:], t1[:], t2[:])
                th = sp.tile([128, CH, B], DT_F32, tag=f"th{dd}")
                nc.scalar.activation(th[:], cf_out[:], AF.Tanh)
                nc.vector.tensor_mul(hout_ap, sif[:, 2 * CH:3 * CH, :], th[:])

            def gate_matmuls(ps, whh_t, xf_ap_fn, hprev_ap_fn):
                for m in range(GM):
                    nc.tensor.matmul(ps[:, m, :], id_sb[:, :], xf_ap_fn(m),
                                     start=True, stop=False)
                    for k in range(CH):
                        nc.tensor.matmul(
                            ps[:, m, :], whh_t[:, k, bass.ts(m, 128)],
                            hprev_ap_fn(k),
                            start=False, stop=(k == CH - 1))

            # ==========================================================
            # SCAN PHASE
            # ==========================================================
            if "S" in phases:
             with tc.tile_pool(name="wS", bufs=1) as wp, \
                  tc.tile_pool(name="sS", bufs=2) as sp, \
                  tc.tile_pool(name="xsS", bufs=2) as xsp, \
                  tc.tile_pool(name="xfS", bufs=2) as xfp, \
                  tc.tile_pool(name="hS", bufs=2) as hp, \
                  tc.tile_pool(name="cxS", bufs=2) as cxp, \
                  tc.tile_pool(name="gemS", bufs=2,
                               space=bass.MemorySpace.PSUM) as gps, \
                  tc.tile_pool(name="cxPS", bufs=2,
                               space=bass.MemorySpace.PSUM) as cxps_pool, \
                  tc.tile_pool(name="gateS", bufs=2,
                               space=bass.MemorySpace.PSUM) as pps:

                whh = {0: load_w(wp, whh_f, "whhf_sb"),
                       1: load_w(wp, whh_b, "whhb_sb")}
                wih = {0: load_w(wp, wih_f, "wihf_sb"),
                       1: load_w(wp, wih_b, "wihb_sb")}
                bia = {0: load_w(wp, bias_f, "bf_sb"),
                       1: load_w(wp, bias_b, "bb_sb")}
                c2t = load_w(wp, cls2T, "c2t_sb")
                xsrc = {0: xT, 1: xTr}
                hdst = {0: HF_D, 1: HB_D}

                cc = {}
                for d in (0, 1):
                    c0 = wp.tile([128, CH, B], DT_F32, tag=f"c0_{d}",
                                 name=f"c0_{d}")
                    c1 = wp.tile([128, CH, B], DT_F32, tag=f"c1_{d}",
                                 name=f"c1_{d}")
                    nc.vector.memset(c0[:], 0.0)
                    cc[d] = [c0, c1]

                xf_tiles = {}
                h_tiles = {}

                def gemm_block_tasks(d, p):
                    tasks = []

                    def t_load():
                        xs = xsp.tile([128, CH, KC], DT_BF, tag=f"xs{d}")
                        nc.sync.dma_start(
                            xs[:], xsrc[d][:, :, bass.ds(p * KC, KC)])
                        xf = xfp.tile([128, GM, KC], DT_BF, tag=f"xf{d}",
                                      name=f"xf{d}_{p}")
                        xf_tiles[(d, p)] = (xs, xf)
                    tasks.append(t_load)

                    for grp in range(GM // 4):
                        def t_grp(grp=grp):
                            xs, xf = xf_tiles[(d, p)]
                            ps = gps.tile([128, 4, KC], DT_F32, tag="gemps")
                            for m in range(4):
                                for k in range(CH):
                                    nc.tensor.matmul(
                                        ps[:, m, :],
                                        wih[d][:, k, bass.ts(grp * 4 + m, 128)],
                                        xs[:, k, :],
                                        start=(k == 0), stop=False)
                                nc.tensor.matmul(
                                    ps[:, m, :],
                                    bia[d][:, bass.ts(grp * 4 + m, 128)],
                                    ones[:, 0:KC],
                                    start=False, stop=True)
                            nc.scalar.copy(
                                xf[:, grp * 4:(grp + 1) * 4, :], ps[:])
                        tasks.append(t_grp)
                    return tasks

                def cx_block_task(q):
                    def t_cx():
                        hf = cxp.tile([128, CH, KC], DT_BF, tag="cxhf")
                        hb = cxp.tile([128, CH, KC], DT_BF, tag="cxhb")
                        nc.sync.dma_start(
                            hf[:], HF_D[:, :, bass.ds(q * KC, KC)])
                        nc.sync.dma_start(
                            hb[:], HB_D[:, :, bass.ds(q * KC, KC)])
                        ps = cxps_pool.tile([2, KC], DT_F32, tag="cxps")
                        for k in range(C2):
                            mvk = hf[:, k, :] if k < CH else hb[:, k - CH, :]
                            nc.tensor.matmul(ps[:], c2t[:, k, :], mvk,
                                             start=(k == 0),
                                             stop=(k == C2 - 1))
                        ot = cxp.tile([2, KC], DT_F32, tag="cxout")
                        nc.vector.tensor_copy(ot[:], ps[:])
                        nc.sync.dma_start(
                            cx_t[:, bass.ds(q * KC, KC)], ot[:])
                    return t_cx

                def emit_step(d, t):
                    p = t // KB
                    j = t % KB
                    _, xf = xf_tiles[(d, p)]
                    if j == 0:
                        h_tiles[(d, p)] = hp.tile(
                            [128, CH, KB, B], DT_BF, tag=f"h{d}",
                            name=f"h{d}_{p}")
                    ht = h_tiles[(d, p)]
                    jj = j if d == 0 else KB - 1 - j
                    if t == 0:
                        hprev = lambda k: zhc[:, k, :]
                    else:
                        if j > 0:
                            hpt = ht
                            jp = j - 1 if d == 0 else KB - j
                        else:
                            hpt = h_tiles[(d, p - 1)]
                            jp = KB - 1 if d == 0 else 0
                        hprev = lambda k: hpt[:, k, jp, :]
                    ps = pps.tile([128, GM, B], DT_F32, tag=f"g{d}")
                    gate_matmuls(ps, whh[d],
                                 lambda m: xf[:, m, bass.ds(j * B, B)], hprev)
                    cprev, cnew = cc[d][t % 2], cc[d][(t + 1) % 2]
                    step_tail(sp, ps, cprev,
                              ht[:, :, bass.ds(jj, 1), :], cnew, d)
                    if j == KB - 1:
                        pblk = p if d == 0 else NKB - 1 - p
                        fl = ht.rearrange("p a b c -> p a (b c)")
                        dma_eng().dma_start(
                            hdst[d][:, :, bass.ds(pblk * KC, KC)], fl[:])

                # prologue: GEMM blocks 0,1 both dirs
                for d in (0, 1):
                    for tsk in gemm_block_tasks(d, 0):
                        tsk()
                pro = []
                for d in (0, 1):
                    pro.extend(gemm_block_tasks(d, 1))
                for tsk in pro:
                    tsk()

                # cx(q) ready when both HF block q and HB block q landed
                cx_ready = {}
                for q in range(NKB):
                    r = max(16 * q + 15, (S - 1) - 16 * q) + 3
                    cx_ready.setdefault(r, []).append(q)

                pending = []
                for t in range(S):
                    if t % KB == 0 and t // KB + 2 < NKB:
                        for d in (0, 1):
                            pending.extend(gemm_block_tasks(d, t // KB + 2))
                    for q in cx_ready.pop(t, []):
                        pending.append(cx_block_task(q))
                    emit_step(0, t)
                    emit_step(1, t)
                    for _ in range(2):
                        if pending:
                            pending.pop(0)()
                for r in sorted(cx_ready):
                    for q in cx_ready[r]:
                        pending.append(cx_block_task(q))
                while pending:
                    pending.pop(0)()

            # ==========================================================
            # DECODE PHASE
            # ==========================================================
            if "D" in phases:
             with tc.tile_pool(name="wD", bufs=1) as wp, \
                  tc.tile_pool(name="sD", bufs=2) as sp, \
                  tc.tile_pool(name="stD", bufs=2) as stp, \
                  tc.tile_pool(name="wst", bufs=3) as wst, \
                  tc.tile_pool(name="xfD", bufs=2) as xfp, \
                  tc.tile_pool(name="hcD", bufs=2) as hcp, \
                  tc.tile_pool(name="mskD", bufs=2) as mp, \
                  tc.tile_pool(name="outD", bufs=2) as op, \
                  tc.tile_pool(name="psA", bufs=1,
                               space=bass.MemorySpace.PSUM) as psA, \
                  tc.tile_pool(name="psB", bufs=1,
                               space=bass.MemorySpace.PSUM) as psB, \
                  tc.tile_pool(name="gateD", bufs=1,
                               space=bass.MemorySpace.PSUM) as pps:

                swhh_t = load_w(wp, swhh, "swhh_sb")
                wwhh_t = load_w(wp, wwhh, "wwhh_sb")
                sb_t = load_w(wp, sbias, "sb_sb")
                wb_t = load_w(wp, wbias, "wb_sb")

                carry = {}
                for d, nm in ((2, "s"), (3, "w")):
                    hA = wp.tile([128, CH, B], DT_BF, tag=f"hA{nm}",
                                 name=f"hA{nm}")
                    hB = wp.tile([128, CH, B], DT_BF, tag=f"hB{nm}",
                                 name=f"hB{nm}")
                    cA = wp.tile([128, CH, B], DT_F32, tag=f"cA{nm}",
                                 name=f"cA{nm}")
                    cB = wp.tile([128, CH, B], DT_F32, tag=f"cB{nm}",
                                 name=f"cB{nm}")
                    nc.vector.memset(hA[:], 0.0)
                    nc.vector.memset(cA[:], 0.0)
                    carry[d] = dict(h=[hA, hB], c=[cA, cB])

                sd_tiles, wi_tiles = {}, {}
                hc_tiles, wh_tiles = {}, {}
                hfb_tiles, keep_tiles, wmsk_tiles = {}, {}, {}

                def big_gemm_tasks(wname, wsrc, bias_t, mv, dst_fn, psp,
                                   pstag, p):
                    """3072 x 1536 x DC GEMM, weight streamed as
                    (k, m-triple) chunks of [128, 384]. The psum tile is
                    bank-padded [128, 3, 512] so each m-tile owns a full
                    2KB bank and the three accumulation groups can stay
                    open together across the streamed k loop."""
                    tasks = []
                    state = {}
                    for g in range(GM // 3):
                        for k in range(C2):
                            def t_k(g=g, k=k):
                                wt = wst.tile([128, 384], DT_BF,
                                              tag=f"{wname}ck",
                                              name=f"{wname}_{p}_{g}_{k}")
                                nc.gpsimd.dma_start(
                                    wt[:], wsrc[:, k, g, :])
                                if k == 0:
                                    state[g] = psp.tile(
                                        [128, 3, 512], DT_F32, tag=pstag,
                                        name=pstag)
                                ps = state[g]
                                for m in range(3):
                                    nc.tensor.matmul(
                                        ps[:, m, 0:DC],
                                        wt[:, bass.ts(m, 128)], mv(k),
                                        start=(k == 0), stop=False)
                            tasks.append(t_k)

                        def t_fin(g=g):
                            ps = state[g]
                            for m in range(3):
                                nc.tensor.matmul(
                                    ps[:, m, 0:DC],
                                    bias_t[:, bass.ts(g * 3 + m, 128)],
                                    ones[:, 0:DC],
                                    start=False, stop=True)
                            dst = dst_fn()
                            nc.scalar.copy(
                                dst[:, g * 3:(g + 1) * 3, :],
                                ps[:, :, 0:DC])
                        tasks.append(t_fin)
                    return tasks

                def sd_block_tasks(p):
                    tasks = []

                    def t_load():
                        hf = stp.tile([128, CH, DC], DT_BF, tag="hfst")
                        hb = stp.tile([128, CH, DC], DT_BF, tag="hbst")
                        nc.sync.dma_start(
                            hf[:], HF_D[:, :, bass.ds(p * DC, DC)])
                        nc.sync.dma_start(
                            hb[:], HB_D[:, :, bass.ds(p * DC, DC)])
                        hfb_tiles[p] = (hf, hb)
                        ke = mp.tile([128, CH, DC], DT_BF, tag="keep")
                        nc.gpsimd.dma_start(
                            ke[:], keep6[:, :, bass.ds(p * DC, DC)])
                        keep_tiles[p] = ke
                        sd_tiles[p] = xfp.tile([128, GM, DC], DT_BF,
                                               tag="sdblk", name=f"sd_{p}")
                    tasks.append(t_load)

                    def mv(k):
                        hf, hb = hfb_tiles[p]
                        return hf[:, k, :] if k < CH else hb[:, k - CH, :]

                    tasks.extend(big_gemm_tasks(
                        "s", swih3, sb_t, mv, lambda: sd_tiles[p],
                        psA, "sdps", p))
                    return [(p, t) for t in tasks]

                def wi_block_tasks(p):
                    def t_load():
                        se = mp.tile([128, CH, DC], DT_BF, tag="sel")
                        iv = mp.tile([128, CH, DC], DT_BF, tag="inv")
                        nc.gpsimd.dma_start(
                            se[:], wsel6[:, :, bass.ds(p * DC, DC)])
                        nc.gpsimd.tensor_scalar(
                            iv[:], se[:], -1.0, 1.0,
                            mybir.AluOpType.mult, mybir.AluOpType.add)
                        wmsk_tiles[p] = (se, iv)
                        wi_tiles[p] = xfp.tile([128, GM, DC], DT_BF,
                                               tag="wiblk", name=f"wi_{p}")

                    def mv(k):
                        fl = hc_tiles[p].rearrange("p a b c -> p a (b c)")
                        return fl[:, k, :]

                    return [(p, t) for t in [t_load] + big_gemm_tasks(
                        "w", wwih3, wb_t, mv, lambda: wi_tiles[p],
                        psB, "wips", p)]

                def emit_dec_step(d, t):
                    p = t // DB
                    j = t % DB
                    xf = sd_tiles[p] if d == 2 else wi_tiles[p]
                    whh_d = swhh_t if d == 2 else wwhh_t
                    if j == 0:
                        if d == 2:
                            hc_tiles[p] = hcp.tile(
                                [128, C2, DB, B], DT_BF, tag="hcblk",
                                name=f"hc_{p}")
                        else:
                            wh_tiles[p] = hcp.tile(
                                [128, CH, DB, B], DT_BF, tag="whblk",
                                name=f"wh_{p}")
                    cr = carry[d]
                    hprev_t = cr["h"][t % 2]
                    if t == 0:
                        hprev = lambda k: zhc[:, k, :]
                    else:
                        hprev = lambda k: hprev_t[:, k, :]
                    ps = pps.tile([128, GM, B], DT_F32, tag=f"g{d}")
                    gate_matmuls(ps, whh_d,
                                 lambda m: xf[:, m, bass.ds(j * B, B)], hprev)
                    cprev = cr["c"][t % 2]
                    hf = sp.tile([128, CH, B], DT_F32, tag=f"hf{d}")
                    cf = sp.tile([128, CH, B], DT_F32, tag=f"cf{d}")
                    step_tail(sp, ps, cprev, hf[:], cf, d)
                    hnext, cnext = cr["h"][(t + 1) % 2], cr["c"][(t + 1) % 2]
                    jb = bass.ds(j * B, B)
                    if d == 2:
                        blk = hc_tiles[p]
                        nc.scalar.copy(blk[:, 0:CH, bass.ds(j, 1), :], hf[:])
                        nc.scalar.copy(blk[:, CH:2 * CH, bass.ds(j, 1), :],
                                       cf[:])
                        ke = keep_tiles[p]
                        nc.vector.tensor_mul(hnext[:], hf[:], ke[:, :, jb])
                        nc.vector.tensor_mul(cnext[:], cf[:], ke[:, :, jb])
                    else:
                        blk = wh_tiles[p]
                        nc.scalar.copy(blk[:, :, bass.ds(j, 1), :], hf[:])
                        se, iv = wmsk_tiles[p]
                        w0h = sp.tile([128, CH, B], DT_F32, tag="w0h")
                        nc.gpsimd.tensor_mul(w0h[:], hprev_t[:], iv[:, :, jb])
                        w0c = sp.tile([128, CH, B], DT_F32, tag="w0c")
                        nc.gpsimd.tensor_mul(w0c[:], cprev[:], iv[:, :, jb])
                        ph = sp.tile([128, CH, B], DT_F32, tag="ph")
                        nc.vector.tensor_mul(ph[:], hf[:], se[:, :, jb])
                        pc = sp.tile([128, CH, B], DT_F32, tag="pc")
                        nc.vector.tensor_mul(pc[:], cf[:], se[:, :, jb])
                        nc.vector.tensor_add(hnext[:], w0h[:], ph[:])
                        nc.vector.tensor_add(cnext[:], w0c[:], pc[:])

                # task queues hold (block, fn); consumers force-drain their
                # producer block's tasks before first read (emission order
                # defines the dataflow direction in Tile deps).
                pending, wi_pending = [], []

                def drain(q, blk):
                    while q and q[0][0] <= blk:
                        q.pop(0)[1]()

                for _, tsk in sd_block_tasks(0):
                    tsk()
                if NDB > 1:
                    pending.extend(sd_block_tasks(1))

                for slot in range(S + LAG):
                    i = slot
                    w = slot - LAG
                    if i < S:
                        if i % DB == 0 and i // DB + 2 < NDB:
                            pending.extend(sd_block_tasks(i // DB + 2))
                        if i % DB == 0:
                            drain(pending, i // DB)
                        emit_dec_step(2, i)
                        if i % DB == DB - 1:
                            wi_pending.extend(wi_block_tasks(i // DB))
                    if 0 <= w < S:
                        if w % DB == 0:
                            drain(wi_pending, w // DB)
                        emit_dec_step(3, w)
                        if w % DB == DB - 1:
                            pw = w // DB
                            flw = wh_tiles[pw].rearrange("p a b c -> p a (b c)")
                            dma_eng().dma_start(
                                WH_D[:, :, bass.ds(pw * DC, DC)], flw[:])
                    budget = 7
                    while budget and (wi_pending or pending):
                        if wi_pending:
                            wi_pending.pop(0)[1]()
                        else:
                            pending.pop(0)[1]()
                        budget -= 1
                while wi_pending or pending:
                    (wi_pending or pending).pop(0)[1]()

            # ==========================================================
            # PHASE C: wh1 classifier from DRAM-streamed wh1 blocks
            # ==========================================================
            if "D" in phases:
             with tc.tile_pool(name="wC", bufs=1) as wpc, \
                  tc.tile_pool(name="sC", bufs=3) as spc, \
                  tc.tile_pool(name="psC", bufs=2,
                               space=bass.MemorySpace.PSUM) as pspc:
                c1t = load_w(wpc, cls1T, "c1t_sb")
                for p in range(NDB):
                    wh = spc.tile([128, CH, DC], DT_BF, tag="whst")
                    nc.sync.dma_start(
                        wh[:], WH_D[:, :, bass.ds(p * DC, DC)])
                    ps = pspc.tile([2, DC], DT_F32, tag="clsps")
                    for k in range(CH):
                        nc.tensor.matmul(ps[:], c1t[:, k, :], wh[:, k, :],
                                         start=(k == 0), stop=(k == CH - 1))
                    ot = spc.tile([2, DC], DT_F32, tag="wcout")
                    nc.vector.tensor_copy(ot[:], ps[:])
                    nc.sync.dma_start(
                        wcls_t[:, bass.ds(p * DC, DC)], ot[:])

    nc.compile()
    return nc


# --------------------------------------------------------------------------
# host-side preparation / assembly
# --------------------------------------------------------------------------

def _gate_perm(H):
    # torch gate order [i, f, g, o] -> ours [i, f, o, g]
    return np.concatenate([np.arange(0, 2 * H),
                           np.arange(3 * H, 4 * H),
                           np.arange(2 * H, 3 * H)])


def _wT_tiles(w, KD):
    M, K = w.shape
    assert K == KD
    wt = np.ascontiguousarray(w.T).reshape(K // 128, 128, M)
    return np.ascontiguousarray(wt.transpose(1, 0, 2)).astype(BF16)


def _mask6(mask_tb, CH):
    S_, B_ = mask_tb.shape
    flat = mask_tb.reshape(-1)
    out = np.broadcast_to(flat[None, None, :], (128, CH, S_ * B_))
    return np.ascontiguousarray(out).astype(BF16)


def prepare_inputs(inputs, S, B, H, ncores):
    CH = H // 128
    perm = _gate_perm(H)
    x = np.asarray(inputs["hidden_state"], np.float32)
    golds = np.asarray(inputs["golds"]).astype(np.int32)
    assert x.shape[0] == ncores * B

    def wT(name, KD):
        return _wT_tiles(np.asarray(inputs[name], np.float32)[perm], KD)

    def bi(name):
        return np.asarray(inputs[name], np.float32)[perm][None, :].astype(BF16)

    shared = dict(
        wih_f=wT("lstm_Wih_f", H), whh_f=wT("lstm_Whh_f", H),
        wih_b=wT("lstm_Wih_b", H), whh_b=wT("lstm_Whh_b", H),
        swih3=np.ascontiguousarray(
            wT("subw_Wih", 2 * H).reshape(128, 2 * H // 128, 8, 384)),
        swhh=wT("subw_Whh", H),
        wwih3=np.ascontiguousarray(
            wT("word_Wih", 2 * H).reshape(128, 2 * H // 128, 8, 384)),
        wwhh=wT("word_Whh", H),
        cls1T=_wT_tiles(np.asarray(inputs["cls_W"], np.float32)[:, :H], H),
        cls2T=_wT_tiles(np.asarray(inputs["cls_W"], np.float32)[:, H:], 2 * H),
        bias_f=bi("lstm_b_f"), bias_b=bi("lstm_b_b"),
        sbias=bi("subw_b"), wbias=bi("word_b"),
        ident=np.eye(128, dtype=BF16),
    )

    in_maps = []
    for c in range(ncores):
        xs = x[c * B:(c + 1) * B]
        xt = xs.transpose(2, 1, 0).reshape(CH, 128, S, B)
        xT = np.ascontiguousarray(
            xt.transpose(1, 0, 2, 3).reshape(128, CH, S * B)).astype(BF16)
        xTr = np.ascontiguousarray(
            xt[:, :, ::-1, :].transpose(1, 0, 2, 3).reshape(
                128, CH, S * B)).astype(BF16)
        g = golds[c * B:(c + 1) * B, 1:]
        m = (g > 0).astype(np.float32).T
        pad = np.zeros((1, B), np.float32)
        keep_p = np.concatenate([1.0 - m, pad], 0)
        sel_p = np.concatenate([m, pad], 0)
        im = dict(shared)
        im.update(xT=xT, xTr=xTr, keep6=_mask6(keep_p, CH),
                  wsel6=_mask6(sel_p, CH))
        in_maps.append(im)

    assembly = dict(cls_b=np.asarray(inputs["cls_b"], np.float32),
                    S=S, B=B, ncores=ncores)
    return in_maps, assembly


def assemble_output(results, assembly):
    S, B, ncores = assembly["S"], assembly["B"], assembly["ncores"]
    cls_b = assembly["cls_b"]
    out = np.empty((ncores * B, S, 2), np.float32)
    for c in range(ncores):
        cx = results[c]["cx_t"].reshape(2, S, B)
        wc = results[c]["wcls_t"].reshape(2, S, B)
        for j in range(2):
            out[c * B:(c + 1) * B, 1:, j] = (
                cx[j, 1:, :] + wc[j, :S - 1, :]).T + cls_b[j]
    out[:, 0, 0] = -1.0
    out[:, 0, 1] = 1.0
    return out


# --------------------------------------------------------------------------
# entry point
# --------------------------------------------------------------------------

_CACHE = {}


def _get_program():
    if "full" not in _CACHE:
        _CACHE["full"] = build_program(FULL["S"], FULL["B"], FULL["H"],
                                       num_devices=FULL["NCORES"])
    return _CACHE["full"]


def run(inputs, trace=False):
    nc = _get_program()
    in_maps, assembly = prepare_inputs(
        inputs, FULL["S"], FULL["B"], FULL["H"], FULL["NCORES"])
    res = run_bass_kernel_spmd(
        nc, in_maps, core_ids=list(range(FULL["NCORES"])), trace=trace)
    out = assemble_output(res.results, assembly)
    return out, res


def kernel(**inputs) -> np.ndarray:
    out, _ = run(inputs, trace=False)
    return out


# revision 9
# speedup vs baseline: 1.0677x; 1.0677x over previous
"""Trainium2 Bass kernel v2 for nn_BertFreezeSegmentor (BiLSTM + stack-decoder).

Restructure vs v1 baseline:
  - Two pipelined phases instead of six serial ones:
      scan phase:   fwd+bwd LSTM scans interleaved per slot, XF/XB gate
                    GEMMs computed block-by-block in the PE gaps, h history
                    streamed to DRAM per 16-step block, CX classifier GEMM
                    computed as soon as both directions' blocks land.
      decode phase: subword+word chains interleaved (word lags LAG steps),
                    SD / WI / cls GEMMs computed block-by-block in the PE
                    gaps from streamed inputs; swih/wwih streamed from DRAM
                    in (quarter, k) chunks so everything fits SBUF.
  - Per-step critical path shortened: the x-projection (incl. bias) is
    added into PSUM with identity-stationary matmuls so the activations
    read PSUM directly; gates reordered to [i, f, o, g] so one Sigmoid
    covers i,f,o; h written directly into history block tiles.

Sharding: pure data parallelism, 8 examples per core on 8 cores.
"""

import numpy as np
import ml_dtypes

import concourse.bass as bass
import concourse.tile as tile
from concourse import bacc, mybir
from concourse.bass_utils import run_bass_kernel_spmd

BF16 = ml_dtypes.bfloat16
DT_BF = mybir.dt.bfloat16
DT_F32 = mybir.dt.float32
AF = mybir.ActivationFunctionType

FULL = dict(S=256, B=8, H=768, NCORES=8)


def build_program(S, B, H, num_devices=8, phases="SD"):
    CH = H // 128           # h chunks (6)
    GM = 4 * H // 128       # gate m-tiles (24)
    C2 = 2 * H // 128       # [h;c] chunks (12)
    NC = S * B
    KB = 16                 # scan block steps
    DB = 32                 # decode block steps
    KC = KB * B
    DC = DB * B
    NKB = S // KB
    NDB = S // DB
    LAG = 40                # word chain lag (steps) behind subword
    assert S % KB == 0 and S % DB == 0

    nc = bacc.Bacc("TRN2", target_bir_lowering=False, debug=False,
                   enable_asserts=False, num_devices=num_devices)

    def inp(name, shape, dt):
        return nc.dram_tensor(name, shape, dt, kind="ExternalInput").ap()

    def scratch(name, shape, dt):
        return nc.dram_tensor(name, shape, dt, kind="Internal").ap()

    def outp(name, shape, dt):
        return nc.dram_tensor(name, shape, dt, kind="ExternalOutput").ap()

    # ---- inputs ----
    xT = inp("xT", [128, CH, NC], DT_BF)
    xTr = inp("xTr", [128, CH, NC], DT_BF)
    wih_f = inp("wih_f", [128, CH, 4 * H], DT_BF)
    whh_f = inp("whh_f", [128, CH, 4 * H], DT_BF)
    wih_b = inp("wih_b", [128, CH, 4 * H], DT_BF)
    whh_b = inp("whh_b", [128, CH, 4 * H], DT_BF)
    bias_f = inp("bias_f", [1, 4 * H], DT_BF)
    bias_b = inp("bias_b", [1, 4 * H], DT_BF)
    swih3 = inp("swih3", [128, C2, 8, 384], DT_BF)   # (k, m-triple) chunks
    swhh = inp("swhh", [128, CH, 4 * H], DT_BF)
    sbiasP = inp("sbiasP", [128, GM], DT_F32)
    wwih3 = inp("wwih3", [128, C2, 8, 384], DT_BF)
    wwhh = inp("wwhh", [128, CH, 4 * H], DT_BF)
    wbiasP = inp("wbiasP", [128, GM], DT_F32)
    cls1T = inp("cls1T", [128, CH, 2], DT_BF)
    cls2T = inp("cls2T", [128, C2, 2], DT_BF)
    keep6 = inp("keep6", [128, CH, NC], DT_BF)
    wsel6 = inp("wsel6", [128, CH, NC], DT_BF)
    ident = inp("ident", [128, 128], DT_BF)

    # ---- DRAM scratch: h histories by t ----
    HF_D = scratch("HF_D", [128, CH, NC], DT_BF)
    HB_D = scratch("HB_D", [128, CH, NC], DT_BF)
    WH_D = scratch("WH_D", [128, CH, NC], DT_BF)

    # ---- outputs ----
    cx_t = outp("cx_t", [2, NC], DT_F32)
    wcls_t = outp("wcls_t", [2, NC], DT_F32)

    with tile.TileContext(nc) as tc:
        _dma_rr = [0]

        def dma_eng():
            _dma_rr[0] += 1
            return nc.sync if _dma_rr[0] % 2 else nc.gpsimd

        def load_w(pool, src, tag):
            t = pool.tile(list(src.shape), src.dtype, tag=tag, name=tag)
            if len(src.shape) >= 3 and src.shape[1] > 1:
                for k in range(src.shape[1]):
                    dma_eng().dma_start(t[:, k], src[:, k])
            else:
                dma_eng().dma_start(t[:], src[:])
            return t

        with tc.tile_pool(name="const", bufs=1) as cpool:
            id_sb = load_w(cpool, ident, "id_sb")
            ones = cpool.tile([1, DC], DT_BF, tag="ones", name="ones")
            nc.vector.memset(ones[:], 1.0)
            zhc = cpool.tile([128, CH, B], DT_BF, tag="zhc", name="zhc")
            nc.vector.memset(zhc[:], 0.0)

            def step_tail(sp, ps, cprev, hout_ap, cf_out, dd):
                """gates psum [128, GM, B] (order i,f,o,g) -> h, c."""
                sif = sp.tile([128, 3 * CH, B], DT_F32, tag=f"sif{dd}")
                nc.scalar.activation(sif[:], ps[:, 0:3 * CH, :], AF.Sigmoid)
                tg = sp.tile([128, CH, B], DT_F32, tag=f"tg{dd}")
                nc.scalar.activation(tg[:], ps[:, 3 * CH:4 * CH, :], AF.Tanh)
                t1 = sp.tile([128, CH, B], DT_F32, tag=f"t1{dd}")
                nc.vector.tensor_mul(t1[:], sif[:, CH:2 * CH, :], cprev[:])
                t2 = sp.tile([128, CH, B], DT_F32, tag=f"t2{dd}")
                nc.vector.tensor_mul(t2[:], sif[:, 0:CH, :], tg[:])
                nc.vector.tensor_add(cf_out, t1[:], t2[:])
                th = sp.tile([128, CH, B], DT_F32, tag=f"th{dd}")
                nc.scalar.activation(th[:], cf_out, AF.Tanh)
                nc.vector.tensor_mul(hout_ap, sif[:, 2 * CH:3 * CH, :], th[:])

            def gate_matmuls(ps, whh_t, xf_ap_fn, hprev_ap_fn):
                for m in range(GM):
                    nc.tensor.matmul(ps[:, m, :], id_sb[:, :], xf_ap_fn(m),
                                     start=True, stop=False)
                    for k in range(CH):
                        nc.tensor.matmul(
                            ps[:, m, :], whh_t[:, k, bass.ts(m, 128)],
                            hprev_ap_fn(k),
                            start=False, stop=(k == CH - 1))

            # ==========================================================
            # SCAN PHASE
            # ==========================================================
            if "S" in phases:
             with tc.tile_pool(name="wS", bufs=1) as wp, \
                  tc.tile_pool(name="sS", bufs=2) as sp, \
                  tc.tile_pool(name="xsS", bufs=2) as xsp, \
                  tc.tile_pool(name="xfS", bufs=2) as xfp, \
                  tc.tile_pool(name="hS", bufs=2) as hp, \
                  tc.tile_pool(name="cxS", bufs=2) as cxp, \
                  tc.tile_pool(name="gemS", bufs=2,
                               space=bass.MemorySpace.PSUM) as gps, \
                  tc.tile_pool(name="cxPS", bufs=2,
                               space=bass.MemorySpace.PSUM) as cxps_pool, \
                  tc.tile_pool(name="gateS", bufs=2,
                               space=bass.MemorySpace.PSUM) as pps:

                whh = {0: load_w(wp, whh_f, "whhf_sb"),
                       1: load_w(wp, whh_b, "whhb_sb")}
                wih = {0: load_w(wp, wih_f, "wihf_sb"),
                       1: load_w(wp, wih_b, "wihb_sb")}
                bia = {0: load_w(wp, bias_f, "bf_sb"),
                       1: load_w(wp, bias_b, "bb_sb")}
                c2t = load_w(wp, cls2T, "c2t_sb")
                xsrc = {0: xT, 1: xTr}
                hdst = {0: HF_D, 1: HB_D}

                cc = {}
                for d in (0, 1):
                    c0 = wp.tile([128, CH, B], DT_F32, tag=f"c0_{d}",
                                 name=f"c0_{d}")
                    c1 = wp.tile([128, CH, B], DT_F32, tag=f"c1_{d}",
                                 name=f"c1_{d}")
                    nc.vector.memset(c0[:], 0.0)
                    cc[d] = [c0, c1]

                xf_tiles = {}
                h_tiles = {}

                def gemm_block_tasks(d, p):
                    tasks = []

                    def t_load():
                        xs = xsp.tile([128, CH, KC], DT_BF, tag=f"xs{d}")
                        nc.sync.dma_start(
                            xs[:], xsrc[d][:, :, bass.ds(p * KC, KC)])
                        xf = xfp.tile([128, GM, KC], DT_BF, tag=f"xf{d}",
                                      name=f"xf{d}_{p}")
                        xf_tiles[(d, p)] = (xs, xf)
                    tasks.append(t_load)

                    for grp in range(GM // 4):
                        def t_grp(grp=grp):
                            xs, xf = xf_tiles[(d, p)]
                            ps = gps.tile([128, 4, KC], DT_F32, tag="gemps")
                            for m in range(4):
                                for k in range(CH):
                                    nc.tensor.matmul(
                                        ps[:, m, :],
                                        wih[d][:, k, bass.ts(grp * 4 + m, 128)],
                                        xs[:, k, :],
                                        start=(k == 0), stop=False)
                                nc.tensor.matmul(
                                    ps[:, m, :],
                                    bia[d][:, bass.ts(grp * 4 + m, 128)],
                                    ones[:, 0:KC],
                                    start=False, stop=True)
                            nc.scalar.copy(
                                xf[:, grp * 4:(grp + 1) * 4, :], ps[:])
                        tasks.append(t_grp)
                    return tasks

                def cx_block_task(q):
                    def t_cx():
                        hf = cxp.tile([128, CH, KC], DT_BF, tag="cxhf")
                        hb = cxp.tile([128, CH, KC], DT_BF, tag="cxhb")
                        nc.sync.dma_start(
                            hf[:], HF_D[:, :, bass.ds(q * KC, KC)])
                        nc.sync.dma_start(
                            hb[:], HB_D[:, :, bass.ds(q * KC, KC)])
                        ps = cxps_pool.tile([2, KC], DT_F32, tag="cxps")
                        for k in range(C2):
                            mvk = hf[:, k, :] if k < CH else hb[:, k - CH, :]
                            nc.tensor.matmul(ps[:], c2t[:, k, :], mvk,
                                             start=(k == 0),
                                             stop=(k == C2 - 1))
                        ot = cxp.tile([2, KC], DT_F32, tag="cxout")
                        nc.vector.tensor_copy(ot[:], ps[:])
                        nc.sync.dma_start(
                            cx_t[:, bass.ds(q * KC, KC)], ot[:])
                    return t_cx

                def emit_step(d, t):
                    p = t // KB
                    j = t % KB
                    _, xf = xf_tiles[(d, p)]
                    if j == 0:
                        h_tiles[(d, p)] = hp.tile(
                            [128, CH, KB, B], DT_BF, tag=f"h{d}",
                            name=f"h{d}_{p}")
                    ht = h_tiles[(d, p)]
                    jj = j if d == 0 else KB - 1 - j
                    if t == 0:
                        hprev = lambda k: zhc[:, k, :]
                    else:
                        if j > 0:
                            hpt = ht
                            jp = j - 1 if d == 0 else KB - j
                        else:
                            hpt = h_tiles[(d, p - 1)]
                            jp = KB - 1 if d == 0 else 0
                        hprev = lambda k: hpt[:, k, jp, :]
                    ps = pps.tile([128, GM, B], DT_F32, tag=f"g{d}")
                    gate_matmuls(ps, whh[d],
                                 lambda m: xf[:, m, bass.ds(j * B, B)], hprev)
                    cprev, cnew = cc[d][t % 2], cc[d][(t + 1) % 2]
                    step_tail(sp, ps, cprev,
                              ht[:, :, bass.ds(jj, 1), :], cnew[:], d)
                    if j == KB - 1:
                        pblk = p if d == 0 else NKB - 1 - p
                        fl = ht.rearrange("p a b c -> p a (b c)")
                        dma_eng().dma_start(
                            hdst[d][:, :, bass.ds(pblk * KC, KC)], fl[:])

                # prologue: GEMM blocks 0,1 both dirs
                for d in (0, 1):
                    for tsk in gemm_block_tasks(d, 0):
                        tsk()
                pro = []
                for d in (0, 1):
                    pro.extend(gemm_block_tasks(d, 1))
                for tsk in pro:
                    tsk()

                # cx(q) ready when both HF block q and HB block q landed
                cx_ready = {}
                for q in range(NKB):
                    r = max(16 * q + 15, (S - 1) - 16 * q) + 3
                    cx_ready.setdefault(r, []).append(q)

                pending = []
                for t in range(S):
                    if t % KB == 0 and t // KB + 2 < NKB:
                        for d in (0, 1):
                            pending.extend(gemm_block_tasks(d, t // KB + 2))
                    for q in cx_ready.pop(t, []):
                        pending.append(cx_block_task(q))
                    emit_step(0, t)
                    emit_step(1, t)
                    for _ in range(2):
                        if pending:
                            pending.pop(0)()
                for r in sorted(cx_ready):
                    for q in cx_ready[r]:
                        pending.append(cx_block_task(q))
                while pending:
                    pending.pop(0)()

            # ==========================================================
            # DECODE PHASE
            # ==========================================================
            if "D" in phases:
             with tc.tile_pool(name="wD", bufs=1) as wp, \
                  tc.tile_pool(name="sD", bufs=2) as sp, \
                  tc.tile_pool(name="stD", bufs=2) as stp, \
                  tc.tile_pool(name="wst", bufs=3) as wst, \
                  tc.tile_pool(name="xfD", bufs=2) as xfp, \
                  tc.tile_pool(name="hcD", bufs=2) as hcp, \
                  tc.tile_pool(name="mskD", bufs=2) as mp, \
                  tc.tile_pool(name="outD", bufs=2) as op, \
                  tc.tile_pool(name="psA", bufs=1,
                               space=bass.MemorySpace.PSUM) as psA, \
                  tc.tile_pool(name="psB", bufs=1,
                               space=bass.MemorySpace.PSUM) as psB, \
                  tc.tile_pool(name="gateD", bufs=1,
                               space=bass.MemorySpace.PSUM) as pps:

                swhh_t = load_w(wp, swhh, "swhh_sb")
                wwhh_t = load_w(wp, wwhh, "wwhh_sb")
                sb_t = load_w(wp, sbiasP, "sb_sb")
                wb_t = load_w(wp, wbiasP, "wb_sb")

                carry = {}
                for d, nm in ((2, "s"), (3, "w")):
                    hA = wp.tile([128, CH, B], DT_BF, tag=f"hA{nm}",
                                 name=f"hA{nm}")
                    hB = wp.tile([128, CH, B], DT_BF, tag=f"hB{nm}",
                                 name=f"hB{nm}")
                    cA = wp.tile([128, CH, B], DT_F32, tag=f"cA{nm}",
                                 name=f"cA{nm}")
                    cB = wp.tile([128, CH, B], DT_F32, tag=f"cB{nm}",
                                 name=f"cB{nm}")
                    nc.vector.memset(hA[:], 0.0)
                    nc.vector.memset(cA[:], 0.0)
                    carry[d] = dict(h=[hA, hB], c=[cA, cB])

                sd_tiles, wi_tiles = {}, {}
                hc_tiles, wh_tiles = {}, {}
                hfb_tiles, keep_tiles, wmsk_tiles = {}, {}, {}

                def big_gemm_tasks(wname, wsrc, bias_t, mv, dst_fn, psp,
                                   pstag, p):
                    """3072 x 1536 x DC GEMM, weight streamed as
                    (k, m-triple) chunks of [128, 384]. The psum tile is
                    bank-padded [128, 3, 512] so each m-tile owns a full
                    2KB bank and the three accumulation groups can stay
                    open together across the streamed k loop."""
                    tasks = []
                    state = {}
                    for g in range(GM // 3):
                        for k in range(C2):
                            def t_k(g=g, k=k):
                                wt = wst.tile([128, 384], DT_BF,
                                              tag=f"{wname}ck",
                                              name=f"{wname}_{p}_{g}_{k}")
                                nc.gpsimd.dma_start(
                                    wt[:], wsrc[:, k, g, :])
                                if k == 0:
                                    state[g] = psp.tile(
                                        [128, 3, 512], DT_F32, tag=pstag,
                                        name=pstag)
                                ps = state[g]
                                for m in range(3):
                                    nc.tensor.matmul(
                                        ps[:, m, 0:DC],
                                        wt[:, bass.ts(m, 128)], mv(k),
                                        start=(k == 0),
                                        stop=(k == C2 - 1))
                            tasks.append(t_k)

                        def t_fin(g=g):
                            ps = state[g]
                            dst = dst_fn()
                            for m in range(3):
                                gm = g * 3 + m
                                nc.scalar.activation(
                                    dst[:, gm, :], ps[:, m, 0:DC],
                                    AF.Identity,
                                    bias=bias_t[:, gm:gm + 1])
                        tasks.append(t_fin)
                    return tasks

                def sd_block_tasks(p):
                    tasks = []

                    def t_load():
                        hf = stp.tile([128, CH, DC], DT_BF, tag="hfst")
                        hb = stp.tile([128, CH, DC], DT_BF, tag="hbst")
                        nc.sync.dma_start(
                            hf[:], HF_D[:, :, bass.ds(p * DC, DC)])
                        nc.sync.dma_start(
                            hb[:], HB_D[:, :, bass.ds(p * DC, DC)])
                        hfb_tiles[p] = (hf, hb)
                        ke = mp.tile([128, CH, DC], DT_BF, tag="keep")
                        nc.gpsimd.dma_start(
                            ke[:], keep6[:, :, bass.ds(p * DC, DC)])
                        keep_tiles[p] = ke
                        sd_tiles[p] = xfp.tile([128, GM, DC], DT_BF,
                                               tag="sdblk", name=f"sd_{p}")
                    tasks.append(t_load)

                    def mv(k):
                        hf, hb = hfb_tiles[p]
                        return hf[:, k, :] if k < CH else hb[:, k - CH, :]

                    tasks.extend(big_gemm_tasks(
                        "s", swih3, sb_t, mv, lambda: sd_tiles[p],
                        psA, "sdps", p))
                    return [(p, t) for t in tasks]

                def wi_block_tasks(p):
                    def t_load():
                        se = mp.tile([128, CH, DC], DT_BF, tag="sel")
                        iv = mp.tile([128, CH, DC], DT_BF, tag="inv")
                        nc.gpsimd.dma_start(
                            se[:], wsel6[:, :, bass.ds(p * DC, DC)])
                        nc.gpsimd.tensor_scalar(
                            iv[:], se[:], -1.0, 1.0,
                            mybir.AluOpType.mult, mybir.AluOpType.add)
                        wmsk_tiles[p] = (se, iv)
                        wi_tiles[p] = xfp.tile([128, GM, DC], DT_BF,
                                               tag="wiblk", name=f"wi_{p}")

                    def mv(k):
                        fl = hc_tiles[p].rearrange("p a b c -> p a (b c)")
                        return fl[:, k, :]

                    return [(p, t) for t in [t_load] + big_gemm_tasks(
                        "w", wwih3, wb_t, mv, lambda: wi_tiles[p],
                        psB, "wips", p)]

                def emit_dec_step(d, t):
                    p = t // DB
                    j = t % DB
                    xf = sd_tiles[p] if d == 2 else wi_tiles[p]
                    whh_d = swhh_t if d == 2 else wwhh_t
                    if j == 0:
                        if d == 2:
                            hc_tiles[p] = hcp.tile(
                                [128, C2, DB, B], DT_BF, tag="hcblk",
                                name=f"hc_{p}")
                        else:
                            wh_tiles[p] = hcp.tile(
                                [128, CH, DB, B], DT_BF, tag="whblk",
                                name=f"wh_{p}")
                    cr = carry[d]
                    hprev_t = cr["h"][t % 2]
                    if t == 0:
                        hprev = lambda k: zhc[:, k, :]
                    else:
                        hprev = lambda k: hprev_t[:, k, :]
                    ps = pps.tile([128, GM, B], DT_F32, tag=f"g{d}")
                    gate_matmuls(ps, whh_d,
                                 lambda m: xf[:, m, bass.ds(j * B, B)], hprev)
                    cprev = cr["c"][t % 2]
                    hcf = sp.tile([128, 2 * CH, B], DT_F32, tag=f"hcf{d}")
                    hfs = hcf[:, 0:CH, :]
                    cfs = hcf[:, CH:2 * CH, :]
                    step_tail(sp, ps, cprev, hfs, cfs, d)
                    hnext, cnext = cr["h"][(t + 1) % 2], cr["c"][(t + 1) % 2]
                    jb = bass.ds(j * B, B)
                    if d == 2:
                        blk = hc_tiles[p]
                        nc.scalar.copy(blk[:, :, bass.ds(j, 1), :], hcf[:])
                        ke = keep_tiles[p]
                        nc.vector.tensor_mul(hnext[:], hfs, ke[:, :, jb])
                        nc.vector.tensor_mul(cnext[:], cfs, ke[:, :, jb])
                    else:
                        blk = wh_tiles[p]
                        nc.scalar.copy(blk[:, :, bass.ds(j, 1), :], hfs)
                        se, iv = wmsk_tiles[p]
                        w0h = sp.tile([128, CH, B], DT_F32, tag="w0h")
                        nc.gpsimd.tensor_mul(w0h[:], hprev_t[:], iv[:, :, jb])
                        w0c = sp.tile([128, CH, B], DT_F32, tag="w0c")
                        nc.gpsimd.tensor_mul(w0c[:], cprev[:], iv[:, :, jb])
                        ph = sp.tile([128, CH, B], DT_F32, tag="ph")
                        nc.vector.tensor_mul(ph[:], hfs, se[:, :, jb])
                        pc = sp.tile([128, CH, B], DT_F32, tag="pc")
                        nc.vector.tensor_mul(pc[:], cfs, se[:, :, jb])
                        nc.vector.tensor_add(hnext[:], w0h[:], ph[:])
                        nc.vector.tensor_add(cnext[:], w0c[:], pc[:])

                # task queues hold (block, fn); consumers force-drain their
                # producer block's tasks before first read (emission order
                # defines the dataflow direction in Tile deps).
                pending, wi_pending = [], []

                def drain(q, blk):
                    while q and q[0][0] <= blk:
                        q.pop(0)[1]()

                for _, tsk in sd_block_tasks(0):
                    tsk()
                if NDB > 1:
                    pending.extend(sd_block_tasks(1))

                for slot in range(S + LAG):
                    i = slot
                    w = slot - LAG
                    if i < S:
                        if i % DB == 0 and i // DB + 2 < NDB:
                            pending.extend(sd_block_tasks(i // DB + 2))
                        if i % DB == 0:
                            drain(pending, i // DB)
                        emit_dec_step(2, i)
                        if i % DB == DB - 1:
                            wi_pending.extend(wi_block_tasks(i // DB))
                    if 0 <= w < S:
                        if w % DB == 0:
                            drain(wi_pending, w // DB)
                        emit_dec_step(3, w)
                        if w % DB == DB - 1:
                            pw = w // DB
                            flw = wh_tiles[pw].rearrange("p a b c -> p a (b c)")
                            dma_eng().dma_start(
                                WH_D[:, :, bass.ds(pw * DC, DC)], flw[:])
                    budget = 7
                    while budget and (wi_pending or pending):
                        if wi_pending:
                            wi_pending.pop(0)[1]()
                        else:
                            pending.pop(0)[1]()
                        budget -= 1
                while wi_pending or pending:
                    (wi_pending or pending).pop(0)[1]()

            # ==========================================================
            # PHASE C: wh1 classifier from DRAM-streamed wh1 blocks
            # ==========================================================
            if "D" in phases:
             with tc.tile_pool(name="wC", bufs=1) as wpc, \
                  tc.tile_pool(name="sC", bufs=3) as spc, \
                  tc.tile_pool(name="psC", bufs=2,
                               space=bass.MemorySpace.PSUM) as pspc:
                c1t = load_w(wpc, cls1T, "c1t_sb")
                for p in range(NDB):
                    wh = spc.tile([128, CH, DC], DT_BF, tag="whst")
                    nc.sync.dma_start(
                        wh[:], WH_D[:, :, bass.ds(p * DC, DC)])
                    ps = pspc.tile([2, DC], DT_F32, tag="clsps")
                    for k in range(CH):
                        nc.tensor.matmul(ps[:], c1t[:, k, :], wh[:, k, :],
                                         start=(k == 0), stop=(k == CH - 1))
                    ot = spc.tile([2, DC], DT_F32, tag="wcout")
                    nc.vector.tensor_copy(ot[:], ps[:])
                    nc.sync.dma_start(
                        wcls_t[:, bass.ds(p * DC, DC)], ot[:])

    nc.compile()
    return nc


# --------------------------------------------------------------------------
# host-side preparation / assembly
# --------------------------------------------------------------------------

def _gate_perm(H):
    # torch gate order [i, f, g, o] -> ours [i, f, o, g]
    return np.concatenate([np.arange(0, 2 * H),
                           np.arange(3 * H, 4 * H),
                           np.arange(2 * H, 3 * H)])


def _wT_tiles(w, KD):
    M, K = w.shape
    assert K == KD
    wt = np.ascontiguousarray(w.T).reshape(K // 128, 128, M)
    return np.ascontiguousarray(wt.transpose(1, 0, 2)).astype(BF16)


def _mask6(mask_tb, CH):
    S_, B_ = mask_tb.shape
    flat = mask_tb.reshape(-1)
    out = np.broadcast_to(flat[None, None, :], (128, CH, S_ * B_))
    return np.ascontiguousarray(out).astype(BF16)


def prepare_inputs(inputs, S, B, H, ncores):
    CH = H // 128
    perm = _gate_perm(H)
    x = np.asarray(inputs["hidden_state"], np.float32)
    golds = np.asarray(inputs["golds"]).astype(np.int32)
    assert x.shape[0] == ncores * B

    def wT(name, KD):
        return _wT_tiles(np.asarray(inputs[name], np.float32)[perm], KD)

    def bi(name):
        return np.asarray(inputs[name], np.float32)[perm][None, :].astype(BF16)

    shared = dict(
        wih_f=wT("lstm_Wih_f", H), whh_f=wT("lstm_Whh_f", H),
        wih_b=wT("lstm_Wih_b", H), whh_b=wT("lstm_Whh_b", H),
        swih3=np.ascontiguousarray(
            wT("subw_Wih", 2 * H).reshape(128, 2 * H // 128, 8, 384)),
        swhh=wT("subw_Whh", H),
        wwih3=np.ascontiguousarray(
            wT("word_Wih", 2 * H).reshape(128, 2 * H // 128, 8, 384)),
        wwhh=wT("word_Whh", H),
        cls1T=_wT_tiles(np.asarray(inputs["cls_W"], np.float32)[:, :H], H),
        cls2T=_wT_tiles(np.asarray(inputs["cls_W"], np.float32)[:, H:], 2 * H),
        bias_f=bi("lstm_b_f"), bias_b=bi("lstm_b_b"),
        sbiasP=np.ascontiguousarray(
            np.asarray(inputs["subw_b"], np.float32)[perm]
            .reshape(-1, 128).T),
        wbiasP=np.ascontiguousarray(
            np.asarray(inputs["word_b"], np.float32)[perm]
            .reshape(-1, 128).T),
        ident=np.eye(128, dtype=BF16),
    )

    in_maps = []
    for c in range(ncores):
        xs = x[c * B:(c + 1) * B]
        xt = xs.transpose(2, 1, 0).reshape(CH, 128, S, B)
        xT = np.ascontiguousarray(
            xt.transpose(1, 0, 2, 3).reshape(128, CH, S * B)).astype(BF16)
        xTr = np.ascontiguousarray(
            xt[:, :, ::-1, :].transpose(1, 0, 2, 3).reshape(
                128, CH, S * B)).astype(BF16)
        g = golds[c * B:(c + 1) * B, 1:]
        m = (g > 0).astype(np.float32).T
        pad = np.zeros((1, B), np.float32)
        keep_p = np.concatenate([1.0 - m, pad], 0)
        sel_p = np.concatenate([m, pad], 0)
        im = dict(shared)
        im.update(xT=xT, xTr=xTr, keep6=_mask6(keep_p, CH),
                  wsel6=_mask6(sel_p, CH))
        in_maps.append(im)

    assembly = dict(cls_b=np.asarray(inputs["cls_b"], np.float32),
                    S=S, B=B, ncores=ncores)
    return in_maps, assembly


def assemble_output(results, assembly):
    S, B, ncores = assembly["S"], assembly["B"], assembly["ncores"]
    cls_b = assembly["cls_b"]
    out = np.empty((ncores * B, S, 2), np.float32)
    for c in range(ncores):
        cx = results[c]["cx_t"].reshape(2, S, B)
        wc = results[c]["wcls_t"].reshape(2, S, B)
        for j in range(2):
            out[c * B:(c + 1) * B, 1:, j] = (
                cx[j, 1:, :] + wc[j, :S - 1, :]).T + cls_b[j]
    out[:, 0, 0] = -1.0
    out[:, 0, 1] = 1.0
    return out


# --------------------------------------------------------------------------
# entry point
# --------------------------------------------------------------------------

_CACHE = {}


def _get_program():
    if "full" not in _CACHE:
        _CACHE["full"] = build_program(FULL["S"], FULL["B"], FULL["H"],
                                       num_devices=FULL["NCORES"])
    return _CACHE["full"]


def run(inputs, trace=False):
    nc = _get_program()
    in_maps, assembly = prepare_inputs(
        inputs, FULL["S"], FULL["B"], FULL["H"], FULL["NCORES"])
    res = run_bass_kernel_spmd(
        nc, in_maps, core_ids=list(range(FULL["NCORES"])), trace=trace)
    out = assemble_output(res.results, assembly)
    return out, res


def kernel(**inputs) -> np.ndarray:
    out, _ = run(inputs, trace=False)
    return out
